# revision 1
# baseline (speedup 1.0000x reference)
"""DANet 3-layer MLP (B=8192, D=2048) on 8 Trainium2 NeuronCores.

Data-parallel: each core computes 1024 rows of the batch; the three
weight matrices are replicated. On-device everything lives in a
transposed layout (features on SBUF partitions) so the contraction dim
of every matmul is the partition dim and activations chain from layer
to layer without transposes; the host transposes x in and z/a out.

Matmuls run as float32r (TF32-like, full PE rate); PSUM accumulates in
fp32 over 16 k-tiles. Per output tile the DVE adds the bias for the z
output while the ACT engine computes tanh(psum + b) straight into the
next layer's activation buffer.
"""

import numpy as np

import concourse.mybir as mybir
import concourse.tile as tile
from concourse import bacc
from concourse.bass_utils import run_bass_kernel_spmd

NCORES = 8
B = 8192
D = 2048
BL = B // NCORES          # 1024 batch rows per core
P = 128                   # partitions
KT = D // P               # 16 contraction tiles
NPANEL = 512              # weight-panel width (n features per panel)
NPB = D // NPANEL         # 4 panels per layer
NSB = NPANEL // P         # 4 output-feature subblocks per panel
MBLK = 512                # moving-operand width (batch cols per matmul)
MT = BL // MBLK           # 2 batch blocks

f32 = mybir.dt.float32
f32r = mybir.dt.float32r
TANH = mybir.ActivationFunctionType.Tanh

W_BUFS = 8                # weight pool slots ([128,4,512] 1MiB each, 8KB/partition)


def build_nc(reps=1):
    """reps>1 repeats the 3-layer pipeline (same I/O; output DMA only on
    the last rep) — a timing probe: wall-clock marginal between reps
    isolates steady-state device time per 3-layer pass."""
    nc = bacc.Bacc()

    xT = nc.dram_tensor("xT", [D, BL], f32, kind="ExternalInput")
    Ws = [nc.dram_tensor(f"W{l}", [D, D], f32, kind="ExternalInput")
          for l in range(3)]
    bs = [nc.dram_tensor(f"b{l}", [D], f32, kind="ExternalInput")
          for l in range(3)]
    zouts = [nc.dram_tensor(f"z{l}T", [D, BL], f32, kind="ExternalOutput")
             for l in range(3)]
    aouts = [nc.dram_tensor(f"a{l}T", [D, BL], f32, kind="ExternalOutput")
             for l in range(3)]

    with tile.TileContext(nc) as tc:
        with (
            tc.tile_pool(name="acts", bufs=1) as actp,
            tc.tile_pool(name="wpool", bufs=W_BUFS) as wpool,
            tc.tile_pool(name="zpool", bufs=4) as zpool,
            tc.tile_pool(name="misc", bufs=1) as misc,
            tc.tile_pool(name="psum", bufs=6, space="PSUM") as psp,
        ):
            # Persistent ping-pong activation buffers, transposed layout:
            # acts[s][k] holds features [128k, 128k+128) x all 1024 batch cols.
            acts = [
                [actp.tile([P, BL], f32r, name=f"act{s}_{k}", tag=f"act{s}_{k}")
                 for k in range(KT)]
                for s in range(2)
            ]

            # All three biases in one [128, 48] tile; column l*16+c holds
            # b_l[128c : 128c+128].
            bias = misc.tile([P, 3 * KT], f32, name="bias", tag="bias")
            for l in range(3):
                nc.sync.dma_start(
                    bias[:, l * KT:(l + 1) * KT],
                    bs[l][:].rearrange("(c p) -> p c", p=P),
                )

            # x -> activation set 0; batch-half m=0 for all k first so the
            # first accumulation chain (which only reads cols 0:512) can
            # start before the second half lands. Layer-0 panel-0 weight
            # loads are interleaved so the first chain's critical DMA set
            # (x-m0 + panel 0) completes as early as possible.
            def load_x(m, k):
                ms = m * MBLK
                nc.sync.dma_start(
                    acts[0][k][:, ms:ms + MBLK],
                    xT[k * P:(k + 1) * P, ms:ms + MBLK].bitcast(f32r))

            def load_w(l, nb, kb, name):
                """One 1 MiB dma_start: 4 k-tiles of a panel -> [128,4,512]."""
                wb = wpool.tile([P, 4, NPANEL], f32r, name=name, tag="wb")
                nc.sync.dma_start(
                    wb[:],
                    Ws[l][kb * 4 * P:(kb + 1) * 4 * P,
                          nb * NPANEL:(nb + 1) * NPANEL]
                    .rearrange("(j p) n -> p j n", p=P)
                    .bitcast(f32r),
                )
                return [wb[:, j, :] for j in range(4)]

            # startup order: interleave x-m0 with panel 0 and x-m1 with
            # panel 1 so the first chains' critical DMA completes earliest.
            first_panels = {0: [], 1: []}
            for kb in range(KT // 4):
                for k in range(kb * 4, kb * 4 + 4):
                    load_x(0, k)
                first_panels[0] += load_w(0, 0, kb, f"w_pre_p0_b{kb}")
            for kb in range(KT // 4):
                for k in range(kb * 4, kb * 4 + 4):
                    load_x(1, k)
                first_panels[1] += load_w(0, 1, kb, f"w_pre_p1_b{kb}")

            # a-output stores are deferred into the next layer's emission:
            # layer l's DMA window is saturated (weights in + z/a out), and
            # the activations persist in SBUF until overwritten two layers
            # later, so shifting the store traffic smooths DMA demand.
            pending_a_stores = []
            for rep in range(reps):
                store = rep == reps - 1
                for l in range(3):
                    li = rep * 3 + l
                    act_in = acts[li % 2]
                    act_out = acts[(li + 1) % 2]
                    flushed_pending = False
                    for nb in range(NPB):
                        if rep == 0 and l == 0 and nb in first_panels:
                            wts = first_panels[nb]
                        else:
                            wts = []
                            for kb in range(KT // 4):
                                wts += load_w(
                                    l, nb, kb, f"w_r{rep}_l{l}_p{nb}_b{kb}")
                        if not flushed_pending:
                            # first panel of this layer is issued; flush the
                            # previous layer's deferred a-stores behind it
                            for emit in pending_a_stores:
                                emit()
                            pending_a_stores = []
                            flushed_pending = True
                        for m in range(MT):
                            ms = m * MBLK
                            for ns in range(NSB):
                                ni = nb * NSB + ns  # output-feature block 0..15
                                bcol = bias[:, l * KT + ni:l * KT + ni + 1]
                                psum = psp.tile([P, MBLK], f32,
                                                name=f"psum_{li}_{ni}_{m}",
                                                tag="psum")
                                for k in range(KT):
                                    nc.tensor.matmul(
                                        psum[:],
                                        wts[k][:, ns * P:(ns + 1) * P],
                                        act_in[k][:, ms:ms + MBLK],
                                        start=(k == 0),
                                        stop=(k == KT - 1),
                                    )
                                z_sb = zpool.tile([P, MBLK], f32,
                                                  name=f"z_{li}_{ni}_{m}",
                                                  tag="z_sb")
                                nc.vector.tensor_scalar_add(z_sb[:], psum[:], bcol)
                                nc.scalar.activation(
                                    act_out[ni][:, ms:ms + MBLK], psum[:], TANH,
                                    bias=bcol, scale=1.0,
                                )
                                if store:
                                    nc.sync.dma_start(
                                        zouts[l][ni * P:(ni + 1) * P, ms:ms + MBLK],
                                        z_sb[:],
                                    )
                                    if l == 2:
                                        # final layer: store halves eagerly so
                                        # only a [128,512] store trails the
                                        # last chain
                                        nc.sync.dma_start(
                                            aouts[l][ni * P:(ni + 1) * P,
                                                     ms:ms + MBLK],
                                            act_out[ni][:, ms:ms + MBLK]
                                            .bitcast(f32),
                                        )
                                    elif m == MT - 1:
                                        # both batch halves of act_out[ni] written
                                        def emit(l=l, ni=ni, t=act_out[ni]):
                                            nc.sync.dma_start(
                                                aouts[l][ni * P:(ni + 1) * P, :],
                                                t[:].bitcast(f32),
                                            )
                                        pending_a_stores.append(emit)

            for emit in pending_a_stores:  # final layer's a-stores
                emit()

    nc.finalize()
    return nc


_NC_CACHE = None


def _get_nc():
    global _NC_CACHE
    if _NC_CACHE is None:
        _NC_CACHE = build_nc()
    return _NC_CACHE


def kernel(x, W0, b0, W1, b1, W2, b2):
    x = np.asarray(x, dtype=np.float32)
    weights = {
        "W0": np.asarray(W0, dtype=np.float32),
        "b0": np.asarray(b0, dtype=np.float32),
        "W1": np.asarray(W1, dtype=np.float32),
        "b1": np.asarray(b1, dtype=np.float32),
        "W2": np.asarray(W2, dtype=np.float32),
        "b2": np.asarray(b2, dtype=np.float32),
    }
    in_maps = []
    for c in range(NCORES):
        xT = np.ascontiguousarray(x[c * BL:(c + 1) * BL, :].T)
        in_maps.append({"xT": xT, **weights})

    res = run_bass_kernel_spmd(_get_nc(), in_maps, core_ids=list(range(NCORES)))

    out = np.empty((6, B, D), dtype=np.float32)
    for c in range(NCORES):
        r = res.results[c]
        rows = slice(c * BL, (c + 1) * BL)
        for l in range(3):
            out[l, rows, :] = r[f"z{l}T"].T
            out[3 + l, rows, :] = r[f"a{l}T"].T
    return out



# revision 14
# speedup vs baseline: 1.1533x; 1.1533x over previous
"""DANet 3-layer MLP (B=8192, D=2048) on 8 Trainium2 NeuronCores.

Data-parallel: each core computes 1024 rows of the batch; the three
weight matrices are replicated. On-device everything lives in a
transposed layout (features on SBUF partitions) so the contraction dim
of every matmul is the partition dim and activations chain from layer
to layer without transposes; the host transposes x in and z/a out.

Mixed precision: weights/x in fp16 except contraction k-tiles 0-1 of
every layer, which run as a single fp8e4m3 DoubleRow matmul (2 k-tiles
per instruction at double rate). PSUM accumulates fp32; bias add +
tanh read fp32 psum. Measured end-to-end rel err ~1.4e-2 vs the 2e-2
gate; outputs stored fp16. DoubleRow operands use the [p, 2, n]
interleaved layout (z += sum_i W8[:,i,:].T @ a8[:,i,:]); the fp8
activations for layers 1/2 are produced by one extra ACT instruction
on the ni<2 drains, and layer 0's come from the host.

Startup is the critical path. DMA issue rate (not bandwidth) limits
it: every dma_start occupies the descriptor generator ~650ns, so the
layer-0/panel-0 "k-major" phase uses a hand-scheduled JIT sequence of
few DMAs — single-k-tile x rows and weight tiles first for the
earliest possible PE start (~4us), multi-k-tile blocks once the
pipeline is ahead — feeding 8 concurrent psum chains (ni 0..3 x m
0..1). The fp8 step runs at the END of each chain so it never gates
startup. A short warmup train of dummy matmuls plus wait-queue
blockers keeps the PE p-state ramp off the real matmul stream, and
the final chain runs as two half-width chains so its drain+store
pipeline starts before the PE stream ends.
"""

import numpy as np

import concourse.mybir as mybir
import concourse.tile as tile
from concourse import bacc
from concourse.bass_utils import run_bass_kernel_spmd

NCORES = 8
B = 8192
D = 2048
BL = B // NCORES          # 1024 batch rows per core
P = 128                   # partitions
KT = D // P               # 16 contraction tiles
KF8 = 2                   # leading k-tiles computed in fp8 DoubleRow
NPANEL = 512              # weight-panel width (n features per panel)
NPB = D // NPANEL         # 4 panels per layer
NSB = NPANEL // P         # 4 output-feature subblocks per panel
MBLK = 512                # moving-operand width (batch cols per matmul)
MT = BL // MBLK           # 2 batch blocks

f32 = mybir.dt.float32
f16 = mybir.dt.float16
f8 = mybir.dt.float8e4
TANH = mybir.ActivationFunctionType.Tanh
DR = mybir.MatmulPerfMode.DoubleRow

W_BUFS = 8                # phase-B weight pool slots ([128,4,512] f16)

N_DUMMY = 8               # PE p-state warmup matmuls
DUMMY_COLS = 256
N_BLOCK = 4               # PE wait-queue blockers


def build_nc(warmup=True):
    nc = bacc.Bacc()

    xT = nc.dram_tensor("xT", [D, BL], f16, kind="ExternalInput")
    x8T = nc.dram_tensor("x8T", [P, KF8, BL], f8, kind="ExternalInput")
    Ws = [nc.dram_tensor(f"W{l}", [D, D], f16, kind="ExternalInput")
          for l in range(3)]
    W8s = [nc.dram_tensor(f"W8_{l}", [P, KF8, D], f8, kind="ExternalInput")
           for l in range(3)]
    biasM = nc.dram_tensor("biasM", [P, 3 * KT], f32, kind="ExternalInput")
    zouts = [nc.dram_tensor(f"z{l}T", [D, BL], f16, kind="ExternalOutput")
             for l in range(3)]
    aouts = [nc.dram_tensor(f"a{l}T", [D, BL], f16, kind="ExternalOutput")
             for l in range(3)]

    with tile.TileContext(nc) as tc:
        with (
            tc.tile_pool(name="acts", bufs=1) as actp,
            tc.tile_pool(name="wk", bufs=1) as wkp,
            tc.tile_pool(name="wpool", bufs=W_BUFS) as wpool,
            tc.tile_pool(name="zpool", bufs=4) as zpool,
            tc.tile_pool(name="misc", bufs=1) as misc,
            tc.tile_pool(name="psum", bufs=8, space="PSUM") as psp,
        ):
            # Persistent ping-pong activation buffers, transposed layout:
            # acts[s][k] holds features [128k, 128k+128) x all 1024 batch
            # cols. Per-k tiles keep dependency tracking fine-grained.
            # k-tiles 0..KF8-1 live in the fp8 act8 tiles instead.
            acts = [
                [actp.tile([P, BL], f16, name=f"act{s}_{k}", tag=f"act{s}_{k}")
                 for k in range(KT)]
                for s in range(2)
            ]
            act8s = [
                actp.tile([P, KF8, BL], f8, name=f"act8_{s}", tag=f"act8_{s}")
                for s in range(2)
            ]
            w8t = [misc.tile([P, KF8, D], f8, name=f"w8_{l}", tag=f"w8_{l}")
                   for l in range(3)]
            bias = misc.tile([P, 3 * KT], f32, name="bias", tag="bias")
            scratch = misc.tile([P, DUMMY_COLS], f16, name="scr", tag="scr")

            # ---- PE p-state warmup -------------------------------------
            # Dummies keep the PE busy from ~0.3us; blockers fill the
            # 4-deep PE wait queue with a dependency on the first x DMA so
            # later matmuls dispatch after the p-state ramp window and get
            # the full 2.4GHz cycle time.
            ps_dummy = psp.tile([P, MBLK], f32, name="ps_dummy", tag="psum")
            if warmup:
                nc.vector.memset(scratch[:], 0.0)
                for i in range(N_DUMMY):
                    nc.tensor.matmul(
                        ps_dummy[:, :DUMMY_COLS], scratch[:, :P],
                        scratch[:, :DUMMY_COLS], start=True, stop=True)

            # ---- phase A DMAs: layer 0 panel 0, hand-scheduled JIT -----
            # DMA issue rate is ~1/650ns; PE consumes a k-step (8 matmuls)
            # per 1707ns. Small transfers first for the earliest PE start,
            # then multi-k blocks. The fp8 operands are only needed at the
            # end of the chains, so they load after the fp16 stream.
            def load_x(k):
                nc.sync.dma_start(
                    acts[0][k][:], xT[k * P:(k + 1) * P, :])

            wk = {k: wkp.tile([P, NPANEL], f16, name=f"wk{k}", tag=f"wk{k}")
                  for k in (2, 3)}

            def load_wk(k):
                nc.sync.dma_start(wk[k][:], Ws[0][k * P:(k + 1) * P, 0:NPANEL])

            def load_wblk(l, nb, kb, name, pool, tag="wb"):
                """4 k-tiles of a panel -> [128,4,512] in one dma_start."""
                wb = pool.tile([P, 4, NPANEL], f16, name=name, tag=tag)
                nc.sync.dma_start(
                    wb[:],
                    Ws[l][kb * 4 * P:(kb + 1) * 4 * P,
                          nb * NPANEL:(nb + 1) * NPANEL]
                    .rearrange("(j p) n -> p j n", p=P),
                )
                return [wb[:, j, :] for j in range(4)]

            load_x(2)
            load_wk(2)
            load_wk(3)
            load_x(3)
            wts_a = [None, None, wk[2][:], wk[3][:]]
            wts_a += load_wblk(0, 0, 1, "wA_b1", wkp, tag="wA_b1")
            for k in range(4, 8):
                load_x(k)
            wts_a += load_wblk(0, 0, 2, "wA_b2", wkp, tag="wA_b2")
            for k in range(8, 12):
                load_x(k)
            wts_a += load_wblk(0, 0, 3, "wA_b3", wkp, tag="wA_b3")
            for k in range(12, KT):
                load_x(k)
            nc.sync.dma_start(act8s[0][:], x8T[:])
            nc.sync.dma_start(w8t[0][:], W8s[0][:])

            if warmup:
                for i in range(N_BLOCK):
                    nc.tensor.matmul(
                        ps_dummy[:, 0:1], scratch[:, :P],
                        acts[0][2][:, 0:1], start=True, stop=True)

            # bias + prefetch of remaining layer-0 panels behind phase A.
            nc.sync.dma_start(bias[:], biasM[:])
            prefetched = {}
            for nb in (1, 2, 3):
                wts = []
                for kb in range(KT // 4):
                    wts += load_wblk(0, nb, kb, f"w_pre_p{nb}_b{kb}", wpool)
                prefetched[(0, nb)] = wts
            nc.sync.dma_start(w8t[1][:], W8s[1][:])
            nc.sync.dma_start(w8t[2][:], W8s[2][:])

            def mm_dr(psum, l, ni, act8_in, clo, chi):
                """fp8 DoubleRow matmul covering k-tiles 0..KF8-1."""
                nc.tensor.matmul(
                    psum[:],
                    w8t[l][:, :, ni * P:(ni + 1) * P],
                    act8_in[:, :, clo:chi],
                    start=False, stop=True, perf_mode=DR,
                )

            # ---- phase A matmuls + drains ------------------------------
            ps_a = [psp.tile([P, MBLK], f32, name=f"psA_{j}", tag="psum")
                    for j in range(8)]

            def mm_a(j, k):
                m, ns = divmod(j, NSB)
                nc.tensor.matmul(
                    ps_a[j][:],
                    wts_a[k][:, ns * P:(ns + 1) * P],
                    acts[0][k][:, m * MBLK:(m + 1) * MBLK],
                    start=(k == KF8), stop=False,
                )

            # k-major for the fp16 k-tiles except the last; then per chain
            # the last fp16 k-tile plus the fp8 DoubleRow step, so early
            # chains stop (and free their psum banks) sooner
            for k in range(KF8, KT - 1):
                for j in range(8):
                    mm_a(j, k)
            for j in range(8):
                mm_a(j, KT - 1)
                m, ns = divmod(j, NSB)
                mm_dr(ps_a[j], 0, ns, act8s[0],
                      m * MBLK, (m + 1) * MBLK)

            pending_a_stores = []

            def drain(l, ni, m, psum, aset_out, act8_out, lo=0, hi=MBLK):
                """Bias-add z (DVE) + tanh into next acts (ACT) + stores."""
                ms = m * MBLK
                w = hi - lo
                bcol = bias[:, l * KT + ni:l * KT + ni + 1]
                z_sb = zpool.tile([P, w], f16, name=f"z_{l}_{ni}_{m}_{lo}",
                                  tag="z_sb")
                nc.vector.tensor_scalar_add(z_sb[:], psum[:, 0:w], bcol)
                nc.scalar.activation(
                    aset_out[ni][:, ms + lo:ms + hi], psum[:, 0:w], TANH,
                    bias=bcol, scale=1.0,
                )
                if l < 2 and ni < KF8:
                    # next layer's fp8 copy of these activations
                    nc.scalar.activation(
                        act8_out[:, ni, ms + lo:ms + hi], psum[:, 0:w], TANH,
                        bias=bcol, scale=1.0,
                    )
                nc.sync.dma_start(
                    zouts[l][ni * P:(ni + 1) * P, ms + lo:ms + hi], z_sb[:])
                if l == 2:
                    # final layer: store halves eagerly so only a small
                    # store trails the last chain
                    nc.sync.dma_start(
                        aouts[l][ni * P:(ni + 1) * P, ms + lo:ms + hi],
                        aset_out[ni][:, ms + lo:ms + hi])
                elif m == MT - 1:
                    def emit(l=l, ni=ni, t=aset_out[ni]):
                        nc.sync.dma_start(
                            aouts[l][ni * P:(ni + 1) * P, :], t[:])
                    pending_a_stores.append(emit)

            for m in range(MT):
                for ns in range(NSB):
                    drain(0, ns, m, ps_a[m * NSB + ns], acts[1], act8s[1])

            # ---- phase B: remaining panels / layers, chain-major -------
            for l in range(3):
                act_in = acts[l % 2]
                act8_in = act8s[l % 2]
                act_out = acts[(l + 1) % 2]
                act8_out = act8s[(l + 1) % 2]
                flushed_pending = False
                for nb in range(NPB):
                    if l == 0 and nb == 0:
                        continue  # phase A covered it
                    if (l, nb) in prefetched:
                        wts = prefetched.pop((l, nb))
                    else:
                        wts = []
                        for kb in range(KT // 4):
                            wts += load_wblk(l, nb, kb, f"w_l{l}_p{nb}_b{kb}",
                                             wpool)
                    if not flushed_pending:
                        # previous layer's deferred a-stores go behind the
                        # first panel's weight loads
                        for emit in pending_a_stores:
                            emit()
                        pending_a_stores.clear()
                        flushed_pending = True
                    for m in range(MT):
                        for ns in range(NSB):
                            ni = nb * NSB + ns
                            last = (l == 2 and nb == NPB - 1
                                    and m == MT - 1 and ns == NSB - 1)
                            # the final chain runs as two half-width
                            # chains so its drain+store pipeline starts
                            # ~1.7us before the PE stream ends
                            splits = ((0, MBLK // 2), (MBLK // 2, MBLK)) \
                                if last else ((0, MBLK),)
                            for lo, hi in splits:
                                psum = psp.tile([P, hi - lo], f32,
                                                name=f"psum_{l}_{ni}_{m}_{lo}",
                                                tag="psum")
                                for k in range(KF8, KT):
                                    nc.tensor.matmul(
                                        psum[:],
                                        wts[k][:, ns * P:(ns + 1) * P],
                                        act_in[k][:, m * MBLK + lo:
                                                  m * MBLK + hi],
                                        start=(k == KF8), stop=False,
                                    )
                                mm_dr(psum, l, ni, act8_in,
                                      m * MBLK + lo, m * MBLK + hi)
                                drain(l, ni, m, psum, act_out, act8_out,
                                      lo, hi)

            for emit in pending_a_stores:  # final layer has none (eager)
                emit()

    nc.finalize()
    return nc


_NC_CACHE = None


def _get_nc():
    global _NC_CACHE
    if _NC_CACHE is None:
        _NC_CACHE = build_nc()
    return _NC_CACHE


def make_in_maps(x, W0, b0, W1, b1, W2, b2):
    """Full fp32 inputs -> per-core input dicts (fp16/fp8 + packed bias)."""
    import ml_dtypes
    f8np = ml_dtypes.float8_e4m3fn

    weights = {f"W{l}": np.ascontiguousarray(w, dtype=np.float16)
               for l, w in ((0, W0), (1, W1), (2, W2))}
    for l, wsrc in ((0, W0), (1, W1), (2, W2)):
        wf = np.asarray(wsrc, np.float32)
        w8 = wf[0:KF8 * P, :].reshape(KF8, P, D).transpose(1, 0, 2)
        weights[f"W8_{l}"] = np.ascontiguousarray(w8).astype(f8np)
    biasM = np.empty((P, 3 * KT), dtype=np.float32)
    for l, b in ((0, b0), (1, b1), (2, b2)):
        # column l*16+c holds b_l[128c : 128c+128]
        biasM[:, l * KT:(l + 1) * KT] = np.asarray(b, np.float32).reshape(KT, P).T
    in_maps = []
    for c in range(NCORES):
        xTc = np.asarray(x[c * BL:(c + 1) * BL, :]).T.astype(np.float16)
        x8 = xTc[0:KF8 * P, :].reshape(KF8, P, BL).transpose(1, 0, 2)
        in_maps.append({"xT": np.ascontiguousarray(xTc),
                        "x8T": np.ascontiguousarray(x8).astype(f8np),
                        "biasM": biasM, **weights})
    return in_maps


def kernel(x, W0, b0, W1, b1, W2, b2):
    in_maps = make_in_maps(x, W0, b0, W1, b1, W2, b2)
    res = run_bass_kernel_spmd(_get_nc(), in_maps, core_ids=list(range(NCORES)))

    out = np.empty((6, B, D), dtype=np.float32)
    for c in range(NCORES):
        r = res.results[c]
        rows = slice(c * BL, (c + 1) * BL)
        for l in range(3):
            out[l, rows, :] = r[f"z{l}T"].T
            out[3 + l, rows, :] = r[f"a{l}T"].T
    return out


# revision 15
# speedup vs baseline: 1.1564x; 1.0026x over previous
"""DANet 3-layer MLP (B=8192, D=2048) on 8 Trainium2 NeuronCores.

Data-parallel: each core computes 1024 rows of the batch; the three
weight matrices are replicated. On-device everything lives in a
transposed layout (features on SBUF partitions) so the contraction dim
of every matmul is the partition dim and activations chain from layer
to layer without transposes; the host transposes x in and z/a out.

Mixed precision: weights/x in fp16 except contraction k-tiles 0-1 of
every layer, which run as a single fp8e4m3 DoubleRow matmul (2 k-tiles
per instruction at double rate). PSUM accumulates fp32; bias add +
tanh read fp32 psum. Measured end-to-end rel err ~1.4e-2 vs the 2e-2
gate; outputs stored fp16. DoubleRow operands use the [p, 2, n]
interleaved layout (z += sum_i W8[:,i,:].T @ a8[:,i,:]); the fp8
activations for layers 1/2 are produced by one extra ACT instruction
on the ni<2 drains, and layer 0's come from the host.

Startup is the critical path. DMA issue rate (not bandwidth) limits
it: every dma_start occupies the descriptor generator ~650ns, so the
layer-0/panel-0 "k-major" phase uses a hand-scheduled JIT sequence of
few DMAs — single-k-tile x rows and weight tiles first for the
earliest possible PE start (~4us), multi-k-tile blocks once the
pipeline is ahead — feeding 8 concurrent psum chains (ni 0..3 x m
0..1). The fp8 step runs at the END of each chain so it never gates
startup. A short warmup train of dummy matmuls plus wait-queue
blockers keeps the PE p-state ramp off the real matmul stream, and
the final chain runs as two half-width chains so its drain+store
pipeline starts before the PE stream ends.
"""

import numpy as np

import concourse.mybir as mybir
import concourse.tile as tile
from concourse import bacc
from concourse.bass_utils import run_bass_kernel_spmd

NCORES = 8
B = 8192
D = 2048
BL = B // NCORES          # 1024 batch rows per core
P = 128                   # partitions
KT = D // P               # 16 contraction tiles
KF8 = 2                   # leading k-tiles computed in fp8 DoubleRow
NPANEL = 512              # weight-panel width (n features per panel)
NPB = D // NPANEL         # 4 panels per layer
NSB = NPANEL // P         # 4 output-feature subblocks per panel
MBLK = 512                # moving-operand width (batch cols per matmul)
MT = BL // MBLK           # 2 batch blocks

f32 = mybir.dt.float32
f16 = mybir.dt.float16
f8 = mybir.dt.float8e4
TANH = mybir.ActivationFunctionType.Tanh
DR = mybir.MatmulPerfMode.DoubleRow

W_BUFS = 8                # phase-B weight pool slots ([128,4,512] f16)

N_DUMMY = 8               # PE p-state warmup matmuls
DUMMY_COLS = 256
N_BLOCK = 4               # PE wait-queue blockers


def build_nc(warmup=True):
    nc = bacc.Bacc()

    xT = nc.dram_tensor("xT", [D, BL], f16, kind="ExternalInput")
    x8T = nc.dram_tensor("x8T", [P, KF8, BL], f8, kind="ExternalInput")
    Ws = [nc.dram_tensor(f"W{l}", [D, D], f16, kind="ExternalInput")
          for l in range(3)]
    W8s = [nc.dram_tensor(f"W8_{l}", [P, KF8, D], f8, kind="ExternalInput")
           for l in range(3)]
    biasM = nc.dram_tensor("biasM", [P, 3 * KT], f32, kind="ExternalInput")
    zouts = [nc.dram_tensor(f"z{l}T", [D, BL], f16, kind="ExternalOutput")
             for l in range(3)]
    aouts = [nc.dram_tensor(f"a{l}T", [D, BL], f16, kind="ExternalOutput")
             for l in range(3)]

    with tile.TileContext(nc) as tc:
        with (
            tc.tile_pool(name="acts", bufs=1) as actp,
            tc.tile_pool(name="wk", bufs=1) as wkp,
            tc.tile_pool(name="wpool", bufs=W_BUFS) as wpool,
            tc.tile_pool(name="zpool", bufs=4) as zpool,
            tc.tile_pool(name="misc", bufs=1) as misc,
            tc.tile_pool(name="psum", bufs=8, space="PSUM") as psp,
        ):
            # Persistent ping-pong activation buffers, transposed layout:
            # acts[s][k] holds features [128k, 128k+128) x all 1024 batch
            # cols. Per-k tiles keep dependency tracking fine-grained.
            # k-tiles 0..KF8-1 live in the fp8 act8 tiles instead.
            acts = [
                [actp.tile([P, BL], f16, name=f"act{s}_{k}", tag=f"act{s}_{k}")
                 for k in range(KT)]
                for s in range(2)
            ]
            act8s = [
                actp.tile([P, KF8, BL], f8, name=f"act8_{s}", tag=f"act8_{s}")
                for s in range(2)
            ]
            w8t = [misc.tile([P, KF8, D], f8, name=f"w8_{l}", tag=f"w8_{l}")
                   for l in range(3)]
            bias = misc.tile([P, 3 * KT], f32, name="bias", tag="bias")
            scratch = misc.tile([P, DUMMY_COLS], f16, name="scr", tag="scr")

            # ---- PE p-state warmup -------------------------------------
            # Dummies keep the PE busy from ~0.3us; blockers fill the
            # 4-deep PE wait queue with a dependency on the first x DMA so
            # later matmuls dispatch after the p-state ramp window and get
            # the full 2.4GHz cycle time.
            ps_dummy = psp.tile([P, MBLK], f32, name="ps_dummy", tag="psum")
            if warmup:
                nc.vector.memset(scratch[:], 0.0)
                for i in range(N_DUMMY):
                    nc.tensor.matmul(
                        ps_dummy[:, :DUMMY_COLS], scratch[:, :P],
                        scratch[:, :DUMMY_COLS], start=True, stop=True)

            # ---- phase A DMAs: layer 0 panel 0, hand-scheduled JIT -----
            # DMA issue rate is ~1/650ns; PE consumes a k-step (8 matmuls)
            # per 1707ns. Small transfers first for the earliest PE start,
            # then multi-k blocks. The fp8 operands are only needed at the
            # end of the chains, so they load after the fp16 stream.
            def load_x(k):
                nc.sync.dma_start(
                    acts[0][k][:], xT[k * P:(k + 1) * P, :])

            wk = {k: wkp.tile([P, NPANEL], f16, name=f"wk{k}", tag=f"wk{k}")
                  for k in (2, 3)}

            def load_wk(k):
                nc.sync.dma_start(wk[k][:], Ws[0][k * P:(k + 1) * P, 0:NPANEL])

            def load_wblk(l, nb, kb, name, pool, tag="wb"):
                """4 k-tiles of a panel -> [128,4,512] in one dma_start."""
                wb = pool.tile([P, 4, NPANEL], f16, name=name, tag=tag)
                nc.sync.dma_start(
                    wb[:],
                    Ws[l][kb * 4 * P:(kb + 1) * 4 * P,
                          nb * NPANEL:(nb + 1) * NPANEL]
                    .rearrange("(j p) n -> p j n", p=P),
                )
                return [wb[:, j, :] for j in range(4)]

            load_x(2)
            load_wk(2)
            load_wk(3)
            load_x(3)
            wts_a = [None, None, wk[2][:], wk[3][:]]
            wts_a += load_wblk(0, 0, 1, "wA_b1", wkp, tag="wA_b1")
            for k in range(4, 8):
                load_x(k)
            wts_a += load_wblk(0, 0, 2, "wA_b2", wkp, tag="wA_b2")
            for k in range(8, 12):
                load_x(k)
            wts_a += load_wblk(0, 0, 3, "wA_b3", wkp, tag="wA_b3")
            for k in range(12, KT):
                load_x(k)
            nc.sync.dma_start(act8s[0][:], x8T[:])
            nc.sync.dma_start(w8t[0][:], W8s[0][:])

            if warmup:
                for i in range(N_BLOCK):
                    nc.tensor.matmul(
                        ps_dummy[:, 0:1], scratch[:, :P],
                        acts[0][2][:, 0:1], start=True, stop=True)

            # bias + prefetch of remaining layer-0 panels behind phase A.
            nc.sync.dma_start(bias[:], biasM[:])
            prefetched = {}
            for nb in (1, 2, 3):
                wts = []
                for kb in range(KT // 4):
                    wts += load_wblk(0, nb, kb, f"w_pre_p{nb}_b{kb}", wpool)
                prefetched[(0, nb)] = wts
            nc.sync.dma_start(w8t[1][:], W8s[1][:])
            nc.sync.dma_start(w8t[2][:], W8s[2][:])

            def mm_dr(psum, l, ni, act8_in, clo, chi):
                """fp8 DoubleRow matmul covering k-tiles 0..KF8-1."""
                nc.tensor.matmul(
                    psum[:],
                    w8t[l][:, :, ni * P:(ni + 1) * P],
                    act8_in[:, :, clo:chi],
                    start=False, stop=True, perf_mode=DR,
                )

            # ---- phase A matmuls + drains ------------------------------
            ps_a = [psp.tile([P, MBLK], f32, name=f"psA_{j}", tag="psum")
                    for j in range(8)]

            def mm_a(j, k):
                m, ns = divmod(j, NSB)
                nc.tensor.matmul(
                    ps_a[j][:],
                    wts_a[k][:, ns * P:(ns + 1) * P],
                    acts[0][k][:, m * MBLK:(m + 1) * MBLK],
                    start=(k == KF8), stop=False,
                )

            # k-major for the fp16 k-tiles except the last two; then per
            # chain the last fp16 k-tiles plus the fp8 DoubleRow step, so
            # early chains stop (and free their psum banks) sooner
            for k in range(KF8, KT - 2):
                for j in range(8):
                    mm_a(j, k)
            for j in range(8):
                mm_a(j, KT - 2)
                mm_a(j, KT - 1)
                m, ns = divmod(j, NSB)
                mm_dr(ps_a[j], 0, ns, act8s[0],
                      m * MBLK, (m + 1) * MBLK)

            pending_a_stores = []

            def drain(l, ni, m, psum, aset_out, act8_out, lo=0, hi=MBLK):
                """Bias-add z (DVE) + tanh into next acts (ACT) + stores."""
                ms = m * MBLK
                w = hi - lo
                bcol = bias[:, l * KT + ni:l * KT + ni + 1]
                z_sb = zpool.tile([P, w], f16, name=f"z_{l}_{ni}_{m}_{lo}",
                                  tag="z_sb")
                nc.vector.tensor_scalar_add(z_sb[:], psum[:, 0:w], bcol)
                nc.scalar.activation(
                    aset_out[ni][:, ms + lo:ms + hi], psum[:, 0:w], TANH,
                    bias=bcol, scale=1.0,
                )
                if l < 2 and ni < KF8:
                    # next layer's fp8 copy of these activations
                    nc.scalar.activation(
                        act8_out[:, ni, ms + lo:ms + hi], psum[:, 0:w], TANH,
                        bias=bcol, scale=1.0,
                    )
                nc.sync.dma_start(
                    zouts[l][ni * P:(ni + 1) * P, ms + lo:ms + hi], z_sb[:])
                if l == 2:
                    # final layer: store halves eagerly so only a small
                    # store trails the last chain
                    nc.sync.dma_start(
                        aouts[l][ni * P:(ni + 1) * P, ms + lo:ms + hi],
                        aset_out[ni][:, ms + lo:ms + hi])
                elif m == MT - 1:
                    def emit(l=l, ni=ni, t=aset_out[ni]):
                        nc.sync.dma_start(
                            aouts[l][ni * P:(ni + 1) * P, :], t[:])
                    pending_a_stores.append(emit)

            for m in range(MT):
                for ns in range(NSB):
                    drain(0, ns, m, ps_a[m * NSB + ns], acts[1], act8s[1])

            # ---- phase B: remaining panels / layers, chain-major -------
            for l in range(3):
                act_in = acts[l % 2]
                act8_in = act8s[l % 2]
                act_out = acts[(l + 1) % 2]
                act8_out = act8s[(l + 1) % 2]
                flushed_pending = False
                for nb in range(NPB):
                    if l == 0 and nb == 0:
                        continue  # phase A covered it
                    if (l, nb) in prefetched:
                        wts = prefetched.pop((l, nb))
                    else:
                        wts = []
                        for kb in range(KT // 4):
                            wts += load_wblk(l, nb, kb, f"w_l{l}_p{nb}_b{kb}",
                                             wpool)
                    if not flushed_pending:
                        # previous layer's deferred a-stores go behind the
                        # first panel's weight loads
                        for emit in pending_a_stores:
                            emit()
                        pending_a_stores.clear()
                        flushed_pending = True
                    for m in range(MT):
                        for ns in range(NSB):
                            ni = nb * NSB + ns
                            last = (l == 2 and nb == NPB - 1
                                    and m == MT - 1 and ns == NSB - 1)
                            # the final chain runs as two half-width
                            # chains so its drain+store pipeline starts
                            # ~1.7us before the PE stream ends
                            splits = ((0, MBLK // 2), (MBLK // 2, MBLK)) \
                                if last else ((0, MBLK),)
                            for lo, hi in splits:
                                psum = psp.tile([P, hi - lo], f32,
                                                name=f"psum_{l}_{ni}_{m}_{lo}",
                                                tag="psum")
                                for k in range(KF8, KT):
                                    nc.tensor.matmul(
                                        psum[:],
                                        wts[k][:, ns * P:(ns + 1) * P],
                                        act_in[k][:, m * MBLK + lo:
                                                  m * MBLK + hi],
                                        start=(k == KF8), stop=False,
                                    )
                                mm_dr(psum, l, ni, act8_in,
                                      m * MBLK + lo, m * MBLK + hi)
                                drain(l, ni, m, psum, act_out, act8_out,
                                      lo, hi)

            for emit in pending_a_stores:  # final layer has none (eager)
                emit()

    nc.finalize()
    return nc


_NC_CACHE = None


def _get_nc():
    global _NC_CACHE
    if _NC_CACHE is None:
        _NC_CACHE = build_nc()
    return _NC_CACHE


def make_in_maps(x, W0, b0, W1, b1, W2, b2):
    """Full fp32 inputs -> per-core input dicts (fp16/fp8 + packed bias)."""
    import ml_dtypes
    f8np = ml_dtypes.float8_e4m3fn

    weights = {f"W{l}": np.ascontiguousarray(w, dtype=np.float16)
               for l, w in ((0, W0), (1, W1), (2, W2))}
    for l, wsrc in ((0, W0), (1, W1), (2, W2)):
        wf = np.asarray(wsrc, np.float32)
        w8 = wf[0:KF8 * P, :].reshape(KF8, P, D).transpose(1, 0, 2)
        weights[f"W8_{l}"] = np.ascontiguousarray(w8).astype(f8np)
    biasM = np.empty((P, 3 * KT), dtype=np.float32)
    for l, b in ((0, b0), (1, b1), (2, b2)):
        # column l*16+c holds b_l[128c : 128c+128]
        biasM[:, l * KT:(l + 1) * KT] = np.asarray(b, np.float32).reshape(KT, P).T
    in_maps = []
    for c in range(NCORES):
        xTc = np.asarray(x[c * BL:(c + 1) * BL, :]).T.astype(np.float16)
        x8 = xTc[0:KF8 * P, :].reshape(KF8, P, BL).transpose(1, 0, 2)
        in_maps.append({"xT": np.ascontiguousarray(xTc),
                        "x8T": np.ascontiguousarray(x8).astype(f8np),
                        "biasM": biasM, **weights})
    return in_maps


def kernel(x, W0, b0, W1, b1, W2, b2):
    in_maps = make_in_maps(x, W0, b0, W1, b1, W2, b2)
    res = run_bass_kernel_spmd(_get_nc(), in_maps, core_ids=list(range(NCORES)))

    out = np.empty((6, B, D), dtype=np.float32)
    for c in range(NCORES):
        r = res.results[c]
        rows = slice(c * BL, (c + 1) * BL)
        for l in range(3):
            out[l, rows, :] = r[f"z{l}T"].T
            out[3 + l, rows, :] = r[f"a{l}T"].T
    return out


# revision 19
# speedup vs baseline: 1.2454x; 1.0770x over previous
"""DANet 3-layer MLP (B=8192, D=2048) on 8 Trainium2 NeuronCores.

Data-parallel: each core computes 1024 rows of the batch; the three
weight matrices are replicated. On-device everything lives in a
transposed layout (features on SBUF partitions) so the contraction dim
of every matmul is the partition dim and activations chain from layer
to layer without transposes; the host transposes x in and z/a out.

Mixed precision: weights/x in fp16 except contraction k-tiles 0-1 of
every layer, which run as a single fp8e4m3 DoubleRow matmul (2 k-tiles
per instruction at double rate). PSUM accumulates fp32; bias add +
tanh read fp32 psum. Measured end-to-end rel err ~1.4e-2 vs the 2e-2
gate; outputs stored fp16. DoubleRow operands use the [p, 2, n]
interleaved layout (z += sum_i W8[:,i,:].T @ a8[:,i,:]); the fp8
activations for layers 1/2 are produced by one extra ACT instruction
on the ni<2 drains, and layer 0's come from the host.

Startup is the critical path. DMA issue rate (not bandwidth) limits
it: every dma_start occupies the descriptor generator ~650ns, so the
layer-0/panel-0 "k-major" phase uses a hand-scheduled JIT sequence of
few DMAs — single-k-tile x rows and weight tiles first for the
earliest possible PE start (~4us), multi-k-tile blocks once the
pipeline is ahead — feeding 8 concurrent psum chains (ni 0..3 x m
0..1). The fp8 step runs at the END of each chain so it never gates
startup. A short warmup train of dummy matmuls plus wait-queue
blockers keeps the PE p-state ramp off the real matmul stream, and
the final chain runs as two half-width chains so its drain+store
pipeline starts before the PE stream ends.
"""

import numpy as np

import concourse.mybir as mybir
import concourse.tile as tile
from concourse import bacc
from concourse.bass_utils import run_bass_kernel_spmd

NCORES = 8
B = 8192
D = 2048
BL = B // NCORES          # 1024 batch rows per core
P = 128                   # partitions
KT = D // P               # 16 contraction tiles
KF8 = 2                   # leading k-tiles computed in fp8 DoubleRow
NPANEL = 512              # weight-panel width (n features per panel)
NPB = D // NPANEL         # 4 panels per layer
NSB = NPANEL // P         # 4 output-feature subblocks per panel
MBLK = 512                # moving-operand width (batch cols per matmul)
MT = BL // MBLK           # 2 batch blocks

f32 = mybir.dt.float32
f16 = mybir.dt.float16
f8 = mybir.dt.float8e4
TANH = mybir.ActivationFunctionType.Tanh
DR = mybir.MatmulPerfMode.DoubleRow

W_BUFS = 6                # phase-B weight pool slots ([128,4,512] f16)

N_DUMMY = 8               # PE p-state warmup matmuls
DUMMY_COLS = 256
N_BLOCK = 4               # PE wait-queue blockers


def build_nc(warmup=True):
    nc = bacc.Bacc()

    xT = nc.dram_tensor("xT", [D, BL], f16, kind="ExternalInput")
    x8T = nc.dram_tensor("x8T", [P, KF8, BL], f8, kind="ExternalInput")
    Ws = [nc.dram_tensor(f"W{l}", [D, D], f16, kind="ExternalInput")
          for l in range(3)]
    W8s = [nc.dram_tensor(f"W8_{l}", [P, KF8, D], f8, kind="ExternalInput")
           for l in range(3)]
    x8cT = nc.dram_tensor("x8cT", [P, 7, 2, 2, BL], f8, kind="ExternalInput")
    W8c0 = nc.dram_tensor("W8c0", [P, 7, 2, 2, D], f8, kind="ExternalInput")
    biasM = nc.dram_tensor("biasM", [P, 3 * KT], f32, kind="ExternalInput")
    zouts = [nc.dram_tensor(f"z{l}T", [D, BL], f16, kind="ExternalOutput")
             for l in range(3)]
    aouts = [nc.dram_tensor(f"a{l}T", [D, BL], f16, kind="ExternalOutput")
             for l in range(3)]

    with tile.TileContext(nc) as tc:
        with (
            tc.tile_pool(name="acts", bufs=1) as actp,
            tc.tile_pool(name="wk", bufs=1) as wkp,
            tc.tile_pool(name="wpool", bufs=W_BUFS) as wpool,
            tc.tile_pool(name="zpool", bufs=12) as zpool,
            tc.tile_pool(name="misc", bufs=1) as misc,
            tc.tile_pool(name="psum", bufs=8, space="PSUM") as psp,
        ):
            # Persistent ping-pong activation buffers, transposed layout:
            # acts[s][k] holds features [128k, 128k+128) x all 1024 batch
            # cols. Per-k tiles keep dependency tracking fine-grained.
            # k-tiles 0..KF8-1 live in the fp8 act8 tiles instead.
            acts = [
                [actp.tile([P, BL], f16, name=f"act{s}_{k}", tag=f"act{s}_{k}")
                 for k in range(KT)]
                for s in range(2)
            ]
            act8s = [
                actp.tile([P, KF8, BL], f8, name=f"act8_{s}", tag=f"act8_{s}")
                for s in range(2)
            ]
            w8t = [misc.tile([P, KF8, D], f8, name=f"w8_{l}", tag=f"w8_{l}")
                   for l in range(3)]
            x8c = [misc.tile([P, 2, 2, BL], f8, name=f"x8c{p}", tag=f"x8c{p}")
                   for p in range(7)]
            w8c = [[None] * NPB for p in range(7)]
            bias = misc.tile([P, 3 * KT], f32, name="bias", tag="bias")
            scratch = misc.tile([P, DUMMY_COLS], f16, name="scr", tag="scr")

            # ---- PE p-state warmup -------------------------------------
            # Dummies keep the PE busy from ~0.3us; blockers fill the
            # 4-deep PE wait queue with a dependency on the first x DMA so
            # later matmuls dispatch after the p-state ramp window and get
            # the full 2.4GHz cycle time.
            ps_dummy = psp.tile([P, MBLK], f32, name="ps_dummy", tag="psum")
            if warmup:
                nc.vector.memset(scratch[:], 0.0)
                for i in range(N_DUMMY):
                    nc.tensor.matmul(
                        ps_dummy[:, :DUMMY_COLS], scratch[:, :P],
                        scratch[:, :DUMMY_COLS], start=True, stop=True)

            # ---- phase A DMAs: layer 0 panel 0, hand-scheduled JIT -----
            # DMA issue rate is ~1/650ns; PE consumes a k-step (8 matmuls)
            # per 1707ns. Small transfers first for the earliest PE start,
            # then multi-k blocks. The fp8 operands are only needed at the
            # end of the chains, so they load after the fp16 stream.
            def load_x(k):
                nc.sync.dma_start(
                    acts[0][k][:], xT[k * P:(k + 1) * P, :])

            def load_wblk(l, nb, kb, name, pool, tag="wb"):
                """4 k-tiles of a panel -> [128,4,512] in one dma_start."""
                wb = pool.tile([P, 4, NPANEL], f16, name=name, tag=tag)
                nc.sync.dma_start(
                    wb[:],
                    Ws[l][kb * 4 * P:(kb + 1) * 4 * P,
                          nb * NPANEL:(nb + 1) * NPANEL]
                    .rearrange("(j p) n -> p j n", p=P),
                )
                return [wb[:, j, :] for j in range(4)]

            def load_x8c(p):
                nc.sync.dma_start(x8c[p][:], x8cT[:, p])

            def load_w8c(p, nb):
                t = wkp.tile([P, 2, 2, NPANEL], f8, name=f"w8c_{p}_{nb}",
                             tag=f"w8c_{p}_{nb}")
                nc.sync.dma_start(
                    t[:], W8c0[:, p, :, :, nb * NPANEL:(nb + 1) * NPANEL])
                w8c[p][nb] = t

            for p in range(7):
                load_x8c(p)
                load_w8c(p, 0)
            nc.sync.dma_start(act8s[0][:], x8T[:])
            nc.sync.dma_start(w8t[0][:], W8s[0][:])

            if warmup:
                for i in range(N_BLOCK):
                    nc.tensor.matmul(
                        ps_dummy[:, 0:1], scratch[:, :P],
                        x8c[0][:, 0, 0, 0:1], start=True, stop=True)

            # bias + prefetch of remaining layer-0 panels behind phase A.
            nc.sync.dma_start(bias[:], biasM[:])
            for nb in (1, 2, 3):
                for p in range(7):
                    load_w8c(p, nb)
            prefetched = {}
            for nb in (0,):
                wts = []
                for kb in range(KT // 4):
                    wts += load_wblk(1, nb, kb, f"w_pre_l1p{nb}_b{kb}", wpool)
                prefetched[(1, nb)] = wts
            nc.sync.dma_start(w8t[1][:], W8s[1][:])
            nc.sync.dma_start(w8t[2][:], W8s[2][:])

            def mm_dr(psum, l, ni, act8_in, clo, chi):
                """fp8 DoubleRow matmul covering k-tiles 0..KF8-1."""
                nc.tensor.matmul(
                    psum[:],
                    w8t[l][:, :, ni * P:(ni + 1) * P],
                    act8_in[:, :, clo:chi],
                    start=False, stop=True, perf_mode=DR,
                )

            def mm_pair(psum, p, nb, ns, clo, chi, start=False, stop=False):
                """3 compensated DR matmuls for pair p (hi*hi, hi*lo, lo*hi)."""
                wt = w8c[p][nb]
                for i, (whl, xhl) in enumerate(((0, 0), (1, 0), (0, 1))):
                    nc.tensor.matmul(
                        psum[:],
                        wt[:, whl, :, ns * P:(ns + 1) * P],
                        x8c[p][:, xhl, :, clo:chi],
                        start=(start and i == 0),
                        stop=(stop and i == 2), perf_mode=DR,
                    )

            # ---- phase A matmuls + drains ------------------------------
            ps_a = [psp.tile([P, MBLK], f32, name=f"psA_{j}", tag="psum")
                    for j in range(8)]

            # pair-major; the last pair plus the k01 step run chain-by-
            # chain so early chains stop (and free psum banks) sooner
            for p in range(6):
                for j in range(8):
                    m, ns = divmod(j, NSB)
                    mm_pair(ps_a[j], p, 0, ns, m * MBLK, (m + 1) * MBLK,
                            start=(p == 0))
            for j in range(8):
                m, ns = divmod(j, NSB)
                mm_pair(ps_a[j], 6, 0, ns, m * MBLK, (m + 1) * MBLK)
                mm_dr(ps_a[j], 0, ns, act8s[0],
                      m * MBLK, (m + 1) * MBLK)

            pending_a_stores = []

            def drain(l, ni, m, psum, aset_out, act8_out, lo=0, hi=MBLK):
                """Bias-add z (DVE) + tanh into next acts (ACT) + stores."""
                ms = m * MBLK
                w = hi - lo
                bcol = bias[:, l * KT + ni:l * KT + ni + 1]
                sc = (1.0 / 64.0) if l == 0 else 1.0
                z_sb = zpool.tile([P, w], f16, name=f"z_{l}_{ni}_{m}_{lo}",
                                  tag="z_sb")
                if l == 0:
                    nc.vector.tensor_scalar(
                        z_sb[:], psum[:, 0:w], sc, bcol,
                        mybir.AluOpType.mult, mybir.AluOpType.add)
                else:
                    nc.vector.tensor_scalar_add(z_sb[:], psum[:, 0:w], bcol)
                nc.scalar.activation(
                    aset_out[ni][:, ms + lo:ms + hi], psum[:, 0:w], TANH,
                    bias=bcol, scale=sc,
                )
                if l < 2 and ni < KF8:
                    # next layer's fp8 copy of these activations
                    nc.scalar.activation(
                        act8_out[:, ni, ms + lo:ms + hi], psum[:, 0:w], TANH,
                        bias=bcol, scale=sc,
                    )
                nc.sync.dma_start(
                    zouts[l][ni * P:(ni + 1) * P, ms + lo:ms + hi], z_sb[:])
                if l == 2:
                    # final layer: store halves eagerly so only a small
                    # store trails the last chain
                    nc.sync.dma_start(
                        aouts[l][ni * P:(ni + 1) * P, ms + lo:ms + hi],
                        aset_out[ni][:, ms + lo:ms + hi])
                elif m == MT - 1:
                    def emit(l=l, ni=ni, t=aset_out[ni]):
                        nc.sync.dma_start(
                            aouts[l][ni * P:(ni + 1) * P, :], t[:])
                    pending_a_stores.append(emit)

            for m in range(MT):
                for ns in range(NSB):
                    drain(0, ns, m, ps_a[m * NSB + ns], acts[1], act8s[1])

            # ---- phase B: remaining panels / layers, chain-major -------
            for l in range(3):
                act_in = acts[l % 2]
                act8_in = act8s[l % 2]
                act_out = acts[(l + 1) % 2]
                act8_out = act8s[(l + 1) % 2]
                flushed_pending = False
                for nb in range(NPB):
                    if l == 0 and nb == 0:
                        continue  # phase A covered it
                    if l == 0:
                        wts = None
                    elif (l, nb) in prefetched:
                        wts = prefetched.pop((l, nb))
                    else:
                        wts = []
                        for kb in range(KT // 4):
                            wts += load_wblk(l, nb, kb, f"w_l{l}_p{nb}_b{kb}",
                                             wpool)
                    if not flushed_pending:
                        # previous layer's deferred a-stores go behind the
                        # first panel's weight loads
                        for emit in pending_a_stores:
                            emit()
                        pending_a_stores.clear()
                        flushed_pending = True
                    for m in range(MT):
                        for ns in range(NSB):
                            ni = nb * NSB + ns
                            last = (l == 2 and nb == NPB - 1
                                    and m == MT - 1 and ns == NSB - 1)
                            # the final chain runs as two half-width
                            # chains so its drain+store pipeline starts
                            # ~1.7us before the PE stream ends
                            splits = ((0, MBLK // 2), (MBLK // 2, MBLK)) \
                                if last else ((0, MBLK),)
                            for lo, hi in splits:
                                psum = psp.tile([P, hi - lo], f32,
                                                name=f"psum_{l}_{ni}_{m}_{lo}",
                                                tag="psum")
                                if l == 0:
                                    for p in range(7):
                                        mm_pair(psum, p, nb, ns,
                                                m * MBLK + lo, m * MBLK + hi,
                                                start=(p == 0))
                                else:
                                    for k in range(KF8, KT):
                                        nc.tensor.matmul(
                                            psum[:],
                                            wts[k][:, ns * P:(ns + 1) * P],
                                            act_in[k][:, m * MBLK + lo:
                                                      m * MBLK + hi],
                                            start=(k == KF8), stop=False,
                                        )
                                mm_dr(psum, l, ni, act8_in,
                                      m * MBLK + lo, m * MBLK + hi)
                                drain(l, ni, m, psum, act_out, act8_out,
                                      lo, hi)

            for emit in pending_a_stores:  # final layer has none (eager)
                emit()

    nc.finalize()
    return nc


_NC_CACHE = None


def _get_nc():
    global _NC_CACHE
    if _NC_CACHE is None:
        _NC_CACHE = build_nc()
    return _NC_CACHE


def make_in_maps(x, W0, b0, W1, b1, W2, b2):
    """Full fp32 inputs -> per-core input dicts (fp16/fp8 + packed bias)."""
    import ml_dtypes
    f8np = ml_dtypes.float8_e4m3fn

    weights = {f"W{l}": np.ascontiguousarray(w, dtype=np.float16)
               for l, w in ((0, W0), (1, W1), (2, W2))}
    for l, wsrc in ((0, W0), (1, W1), (2, W2)):
        wf = np.asarray(wsrc, np.float32) * (64.0 if l == 0 else 1.0)
        w8 = wf[0:KF8 * P, :].reshape(KF8, P, D).transpose(1, 0, 2)
        weights[f"W8_{l}"] = np.ascontiguousarray(w8).astype(f8np)
    biasM = np.empty((P, 3 * KT), dtype=np.float32)
    for l, b in ((0, b0), (1, b1), (2, b2)):
        # column l*16+c holds b_l[128c : 128c+128]
        biasM[:, l * KT:(l + 1) * KT] = np.asarray(b, np.float32).reshape(KT, P).T
    # layer-0 compensated pairs (k-tiles 2..15): hi = f8(W), lo = f8(W-hi)
    w0f = 64.0 * np.asarray(W0, np.float32)[KF8 * P:, :].reshape(7, 2, P, D)
    whi = w0f.astype(f8np)
    wlo = (w0f - whi.astype(np.float32)).astype(f8np)
    weights["W8c0"] = np.ascontiguousarray(
        np.stack([whi, wlo], axis=1).transpose(3, 0, 1, 2, 4))

    in_maps = []
    for c in range(NCORES):
        xTc = np.asarray(x[c * BL:(c + 1) * BL, :]).T.astype(np.float16)
        x8 = xTc[0:KF8 * P, :].reshape(KF8, P, BL).transpose(1, 0, 2)
        xcf = xTc[KF8 * P:, :].astype(np.float32).reshape(7, 2, P, BL)
        xhi = xcf.astype(f8np)
        xlo = (xcf - xhi.astype(np.float32)).astype(f8np)
        x8c = np.stack([xhi, xlo], axis=1).transpose(3, 0, 1, 2, 4)
        in_maps.append({"xT": np.ascontiguousarray(xTc),
                        "x8T": np.ascontiguousarray(x8).astype(f8np),
                        "x8cT": np.ascontiguousarray(x8c),
                        "biasM": biasM, **weights})
    return in_maps


def kernel(x, W0, b0, W1, b1, W2, b2):
    in_maps = make_in_maps(x, W0, b0, W1, b1, W2, b2)
    res = run_bass_kernel_spmd(_get_nc(), in_maps, core_ids=list(range(NCORES)))

    out = np.empty((6, B, D), dtype=np.float32)
    for c in range(NCORES):
        r = res.results[c]
        rows = slice(c * BL, (c + 1) * BL)
        for l in range(3):
            out[l, rows, :] = r[f"z{l}T"].T
            out[3 + l, rows, :] = r[f"a{l}T"].T
    return out
